# revision 1
# baseline (speedup 1.0000x reference)
"""MoE routed expert matmul on 8 Trainium2 NeuronCores.

Problem: out[n] = input[n] @ w[inds[n]] + b[inds[n]]
  input [262144, 32] f32, inds [262144] i32 (1024 experts), w [1024, 32, 32], b [1024, 1, 32]

Strategy (K-stacked expert quads; host does routing/layout only — all FLOPs
on device):
  * Host sorts the 1024 experts by global token count (ascending) and chunks
    them into 32 quad-groups of 32 experts with near-equal counts.  Chunk q
    supplies one expert to each (core, band) pair: expert chunks[q][4k + r]
    goes to core k, quad q, band r (r in 0..3).  Every core runs the same
    program over its own 32 quads; quad q's column width Q[q] = max token
    count in the chunk (global max, so the SPMD shapes match), rounded up to
    2.  Count-matched chunks keep padding to ~2%.
  * Activation layout xt [128, TOTW] fp16: token t of (quad q, band r) sits
    at column X[q] + t, rows 32r..32r+32 (its 32 features).  Each column
    carries up to 4 tokens (one per band) — full 128-row density.
  * Weights upload as block-diagonal K=64 stacks (wq, ~0.5 MB): for each
    quad and half h, a [64, 64] tile holds experts (q, 2h) and (q, 2h+1) on
    the diagonal.  Two [K=64, M=64, N=Q] matmuls per quad (tile_position
    (0,0) / (64,64)) then compute all 4 bands' tokens — each activation
    column streams through the PE twice instead of 4x (vs per-expert 32x32
    tiles), and the off-diagonal zeros kill the cross-expert terms.  One
    small early group (K32_GROUPS) instead runs 4 per-expert K=32 matmuls,
    halving its weight bytes where the PE has load-phase slack.
  * Matmuls accumulate into pair-level PSUM tiles (2 quads x 512-col banks,
    4 in flight) so the bias latency stays out of the PSUM-recycle loop.
    Bias + fp16 down-convert runs split across engines per 4-quad group:
    ScalarE handles quads 0-1 as two exact-width activation ops, VectorE
    quads 2-3 in one tensor_tensor with a broadcast bias view.
  * Schedule: all xt tiles are SBUF-resident; loads issue upfront on the SP
    ring and run back-to-back, stores queue behind them (2 early groups on
    the GpSimd SWDGE ring, the rest on SP), so the DMA engines never idle
    and the compute tail hides inside the store backlog.  Group processing
    order pulls two small groups early so the drain-phase bias backlog
    never paces the final stores.  fp16 I/O halves DMA traffic vs f32;
    per-core HBM bytes ~4.8 MB -> ~13.5 us at 360 B/ns, sim 17.1 us.
  * Host scatters the sorted outputs back to original token order.

Layouts (core k, quad q, band r = 2h + s, expert e = chunks[q][4k + r]):
  xt [128, TOTW]  xt[32r+i, X[q] + t]        = x[token t of e, feat i]  (fp16)
  wq [128, 2048]  wq[64h+32s+i, 64q+32s+o]   = w[e, i, o], 0 off-diag   (fp16)
  bp [128, 32]    bp[32r+o, q]               = b[e, 0, o]               (fp16)
  ot [128, TOTW]  ot[32r+o, X[q] + t]        = out[token t of e, feat o](fp16)
"""

import numpy as np

import concourse.bass as bass
import concourse.mybir as mybir
import concourse.tile as tile
from concourse import bacc
from concourse.bass_utils import run_bass_kernel_spmd

N_TOK = 262144
E = 1024
F = 32
O = 32
NCORES = 8
NQUAD = 32  # quads per core; 4 experts each = 128 experts/core
GQ = 4  # quads per load/store group
NG = NQUAD // GQ
F32 = mybir.dt.float32
MM_DT = mybir.dt.float16
OT_DT = mybir.dt.float16

N_WARM = 6  # PE ramp warm-up matmuls
WARM_N = 160  # free-dim length of each warm-up matmul
STORE_GPSIMD_N = 1  # leading store groups on the Pool SWDGE ring (rest: SP)
SPLIT_LAST = True  # split the last group's load at its final quad
# group processing order (indices into size-descending groups): two small
# groups early so the engine bias backlog never paces the store drain
GROUP_ORDER = (0, 1, 2, 3, 4, 5, 6, 7)
# processed-group indices whose quads run as 4 per-expert K=32 matmuls
# (half the weight-upload bytes, double the PE rows) — small early groups
# where the PE has load-phase slack
K32_GROUPS = ()
# processed groups whose activations upload as int8 (global scale folded
# into their wq blocks; int8->fp16 cast is exact for |v|<=127). Casts run
# as ONE full-group op each, all emitted BEFORE any bias op in the
# engines' in-order queues, so they fill load-phase idle without coupling
# into the bias cadence. Early groups only: their longer first-chain
# hides in the store backlog.
INT8_GROUPS = (0, 1, 2, 3)

_programs: dict[tuple, "bacc.Bacc"] = {}


class _CapacityOverflow(Exception):
    """A single expert got >512 tokens (~16 sigma out for uniform routing at
    256 tokens/expert).  Handled by a host fallback so kernel() still
    returns a correct result."""


def _plan(counts):
    """Chunk experts into count-matched quads; per-quad widths and offsets."""
    order_e = np.argsort(counts, kind="stable")  # ascending counts
    # chunk q holds 32 count-matched experts; descending so the pipeline
    # tail (last-stored groups) drains on the smallest transfers
    chunks = order_e.reshape(NQUAD, 32)[::-1]
    # optional group-level processing permutation (load/compute/store order)
    chunks = chunks.reshape(NG, GQ, 32)[list(GROUP_ORDER)].reshape(NQUAD, 32)
    Q = np.maximum(16, ((counts[chunks[:, -1]] + 1) // 2) * 2)  # [NQUAD]
    if Q.max() > 512:
        raise _CapacityOverflow(int(counts.max()))
    X = np.zeros(NQUAD + 1, dtype=np.int64)
    np.cumsum(Q, out=X[1:])
    TOTW = int(X[-1])
    j = np.arange(32)
    e_quad = np.empty(E, dtype=np.int64)
    e_core = np.empty(E, dtype=np.int64)
    e_band = np.empty(E, dtype=np.int64)
    e_quad[chunks] = np.arange(NQUAD)[:, None]
    e_core[chunks] = (j // 4)[None, :]
    e_band[chunks] = (j % 4)[None, :]
    k32 = np.zeros(NQUAD, dtype=bool)
    for g in K32_GROUPS:
        k32[GQ * g : GQ * (g + 1)] = True
    wqw = np.where(k32, 32, 64)
    wqX = np.zeros(NQUAD + 1, dtype=np.int64)
    np.cumsum(wqw, out=wqX[1:])
    return Q.astype(np.int64), X, TOTW, e_quad, e_core, e_band, k32, wqX


def _group_cols(X):
    """Per-group widths and per-dtype source-column offsets (int8 groups
    pack into xt8, fp16 groups into xt16, in processed order)."""
    gw = [int(X[GQ * (g + 1)] - X[GQ * g]) for g in range(NG)]
    o8, o16 = [0] * NG, [0] * NG
    c8 = c16 = 0
    for g in range(NG):
        if g in INT8_GROUPS:
            o8[g] = c8
            c8 += gw[g]
        else:
            o16[g] = c16
            c16 += gw[g]
    return gw, o8, o16, c8, c16


def _build(Q, X, TOTW, k32, wqX) -> "bacc.Bacc":
    WQW = int(wqX[-1])
    gw, o8, o16, W8, W16 = _group_cols(X)
    nc = bacc.Bacc("TRN2", target_bir_lowering=False, debug=False, num_devices=NCORES)
    xt8 = nc.declare_dram_parameter("xt8", [128, max(W8, 8)], mybir.dt.int8, isOutput=False)
    xt16 = nc.declare_dram_parameter("xt16", [128, max(W16, 8)], MM_DT, isOutput=False)
    wq = nc.declare_dram_parameter("wq", [128, WQW], MM_DT, isOutput=False)
    bp = nc.declare_dram_parameter("bp", [128, NQUAD], MM_DT, isOutput=False)
    ot = nc.declare_dram_parameter("ot", [128, TOTW], OT_DT, isOutput=True)

    with tile.TileContext(nc) as tc:
        with (
            tc.tile_pool(name="const", bufs=1) as c_pool,
            tc.tile_pool(name="xq", bufs=1) as xq_pool,
            tc.tile_pool(name="xt", bufs=NG) as xt_pool,
            tc.tile_pool(name="out", bufs=NG) as out_pool,
            tc.tile_pool(name="psm", bufs=8, space="PSUM") as psm_pool,
        ):
            wq_t = c_pool.tile([128, WQW], MM_DT)
            bp_t = c_pool.tile([128, NQUAD], MM_DT)
            warm_t = c_pool.tile([128, WARM_N], MM_DT)

            # loads: gpsimd (SWDGE, 25ns SEQ issue) carries wq in chunks —
            # groups 0-1 first so compute starts early — plus the bias;
            # sync (SP HWDGE) carries all xt
            wq2g = int(wqX[2 * GQ])
            nc.gpsimd.dma_start(out=wq_t[:, :wq2g], in_=wq[:, :wq2g])
            nc.gpsimd.dma_start(out=bp_t[:], in_=bp[:])
            nc.gpsimd.dma_start(out=wq_t[:, wq2g:], in_=wq[:, wq2g:])

            # PE ramp warm-up on a memset scratch tile (PSUM never read);
            # the dummy activation pulls ACT's 1.3us LoadActFuncSet into the
            # load phase instead of stalling the first real bias op
            nc.vector.memset(warm_t[:], 0.0)
            nc.scalar.activation(
                warm_t[0:1, 0:1],
                warm_t[0:1, 0:1],
                mybir.ActivationFunctionType.Identity,
                bias=warm_t[0:1, 1:2],
                scale=1.0,
            )
            warm_ps = psm_pool.tile(
                [128, WARM_N], F32, space="PSUM", name="warm_ps", tag="psm"
            )
            for _ in range(N_WARM):
                nc.tensor.matmul(
                    out=warm_ps[0:32, :],
                    lhsT=warm_t[0:32, 0:32],
                    rhs=warm_t[0:32, :],
                    start=True,
                    stop=True,
                    tile_position=(0, 0),
                )

            xt_tiles = {}
            o_tiles = {}
            xq_t = xq_pool.tile([128, max(W8, 8)], mybir.dt.int8)

            def load_group(g, cuts=()):
                a, bnd = int(X[GQ * g]), int(X[GQ * (g + 1)])
                t = xt_pool.tile([128, bnd - a], MM_DT, name="xt_t", tag="xt_t")
                base = o16[g]
                for c0, c1 in zip((a, *cuts), (*cuts, bnd)):
                    nc.sync.dma_start(
                        out=t[:, c0 - a : c1 - a],
                        in_=xt16[:, base + c0 - a : base + c1 - a],
                    )
                xt_tiles[g] = t

            def cast_group(g, eng):
                W = gw[g]
                t = xt_pool.tile([128, W], MM_DT, name="xt_t", tag="xt_t")
                b0 = o8[g]
                if eng == "dve":
                    nc.vector.tensor_scalar_add(t[:], xq_t[:, b0 : b0 + W], 0.0)
                elif eng == "pool":
                    nc.gpsimd.tensor_scalar_add(t[:], xq_t[:, b0 : b0 + W], 0.0)
                elif eng == "split":
                    ca, cb = W * 13 // 25, W * 18 // 25
                    nc.vector.tensor_scalar_add(
                        t[:, :ca], xq_t[:, b0 : b0 + ca], 0.0
                    )
                    nc.scalar.activation(
                        t[:, ca:cb], xq_t[:, b0 + ca : b0 + cb],
                        mybir.ActivationFunctionType.Copy,
                    )
                    nc.gpsimd.tensor_scalar_add(
                        t[:, cb:], xq_t[:, b0 + cb : b0 + W], 0.0
                    )
                else:
                    nc.scalar.activation(
                        t[:], xq_t[:, b0 : b0 + W],
                        mybir.ActivationFunctionType.Copy,
                    )
                xt_tiles[g] = t

            # all xt tiles are resident; loads issue upfront and run
            # back-to-back so stores queue behind them and the compute tail
            # hides inside the store backlog
            # int8 groups load first (tiny transfers), then fp16 groups;
            # all tiles are resident so stores queue behind the loads
            for g in sorted(INT8_GROUPS):
                nc.sync.dma_start(
                    out=xq_t[:, o8[g] : o8[g] + gw[g]],
                    in_=xt8[:, o8[g] : o8[g] + gw[g]],
                )
            fp16_gs = [g for g in range(NG) if g not in INT8_GROUPS]
            for g in fp16_gs[:-1]:
                load_group(g)
            # keep the trailing piece >= 256 cols (512B rows) so it doesn't
            # pay the sub-512B descriptor latency penalty
            gl = fp16_gs[-1]
            a_l, b_l = int(X[GQ * gl]), int(X[GQ * (gl + 1)])
            last_cut = min(b_l - int(Q[GQ * (gl + 1) - 1]), b_l - 256)
            split_ok = SPLIT_LAST and gl == NG - 1 and last_cut > a_l
            load_group(gl, cuts=(last_cut,) if split_ok else ())

            # upfront full-group casts spread over all three cast-capable
            # engines (Pool takes the 3rd: it only has DMA preps otherwise);
            # a 4th int8 group's cast splits three ways into the engines'
            # remaining idle
            engs = ("dve", "act", "pool", "split", "split", "split")
            for i, g in enumerate(sorted(INT8_GROUPS)):
                cast_group(g, engs[i])

            for g in range(NG):
                a, bnd = int(X[GQ * g]), int(X[GQ * (g + 1)])
                o_t = out_pool.tile([128, bnd - a], OT_DT, name="o_t", tag="o_t")
                # quad-level single-bank PSUM tiles (8 in flight): biases
                # track their own quad's mains with ~0.65us lag vs 1.7us of
                # 8-quad lead, so the PSUM recycle never stalls the PE and
                # the drain cadence is store-transfer-, not chain-, paced.
                # Bias + fp16 down-convert alternates ACT / DVE per quad.
                for qi in range(GQ):
                    q = GQ * g + qi
                    Qq = int(Q[q])
                    off = int(X[q] - a)
                    psm = psm_pool.tile(
                        [128, 512], F32, space="PSUM", name="psm", tag="psm"
                    )
                    wx = int(wqX[q])
                    if k32[q]:
                        for r in range(4):
                            nc.tensor.matmul(
                                out=psm[32 * r : 32 * r + 32, :Qq],
                                lhsT=wq_t[32 * r : 32 * r + 32, wx : wx + 32],
                                rhs=xt_tiles[g][
                                    32 * r : 32 * r + 32, off : off + Qq
                                ],
                                start=True,
                                stop=True,
                                tile_position=(32 * r, 32 * r),
                            )
                    else:
                        for h in range(2):
                            nc.tensor.matmul(
                                out=psm[64 * h : 64 * h + 64, :Qq],
                                lhsT=wq_t[64 * h : 64 * h + 64, wx : wx + 64],
                                rhs=xt_tiles[g][
                                    64 * h : 64 * h + 64, off : off + Qq
                                ],
                                start=True,
                                stop=True,
                                tile_position=(64 * h, 64 * h),
                            )
                    if qi % 2 == 0:
                        nc.scalar.activation(
                            o_t[:, off : off + Qq],
                            psm[:, :Qq],
                            mybir.ActivationFunctionType.Identity,
                            bias=bp_t[:, q : q + 1],
                            scale=1.0,
                        )
                    else:
                        nc.vector.tensor_tensor(
                            out=o_t[:, off : off + Qq],
                            in0=psm[:, :Qq],
                            in1=bp_t[:, q : q + 1].to_broadcast([128, Qq]),
                            op=mybir.AluOpType.add,
                        )
                # stores: leading groups on the Pool SWDGE ring (25ns SEQ
                # issue, never blocks compute); the rest on the SP ring where
                # they queue behind the loads (loads keep DMA priority) and
                # get the lower-latency HWDGE issue path for the tail
                ring = nc.gpsimd if g < STORE_GPSIMD_N else nc.sync
                ring.dma_start(out=ot[:, a:bnd], in_=o_t[:])

    nc.compile()
    return nc


def _pack(x, inds, w, b):
    """Host-side routing: sort tokens by expert, build per-core device arrays."""
    counts = np.bincount(inds, minlength=E)
    Q, X, TOTW, e_quad, e_core, e_band, k32, wqX = _plan(counts)

    order = np.argsort(inds, kind="stable")
    sorted_inds = inds[order]
    starts = np.zeros(E, dtype=np.int64)
    np.cumsum(counts[:-1], out=starts[1:])
    slot = np.arange(N_TOK, dtype=np.int64) - starts[sorted_inds]

    k_tok = e_core[sorted_inds]
    r_tok = e_band[sorted_inds]
    col_tok = X[e_quad[sorted_inds]] + slot

    mdt = mybir.dt.np(MM_DT)
    s_q = max(float(np.abs(x).max()) / 127.0, 1e-30)
    xt_all = np.zeros((NCORES, 4, F, TOTW), dtype=mdt)
    xt_all[k_tok, r_tok, :, col_tok] = x[order].astype(mdt)
    xta = xt_all.reshape(NCORES, 128, TOTW)
    gw, o8, o16, W8, W16 = _group_cols(X)
    xt8 = np.zeros((NCORES, 128, max(W8, 8)), dtype=np.int8)
    xt16 = np.zeros((NCORES, 128, max(W16, 8)), dtype=mdt)
    for g in range(NG):
        a = int(X[GQ * g])
        sl = xta[:, :, a : a + gw[g]]
        if g in INT8_GROUPS:
            xt8[:, :, o8[g] : o8[g] + gw[g]] = np.clip(
                np.rint(sl.astype(np.float32) / s_q), -127, 127
            ).astype(np.int8)
        else:
            xt16[:, :, o16[g] : o16[g] + gw[g]] = sl

    # per-quad weight blocks: K=64 quads get two [64, 64] diagonal tiles
    # (partition halves), K=32 quads one [32, 32] block per band
    WQW = int(wqX[-1])
    wqk = np.zeros((NCORES, 128, WQW), dtype=mdt)
    wf = w.astype(mdt)
    wfs = (w * s_q).astype(mdt)
    for e in range(E):
        k, q, r = int(e_core[e]), int(e_quad[e]), int(e_band[e])
        wx = int(wqX[q])
        wsrc = wfs if (q // GQ) in INT8_GROUPS else wf
        if k32[q]:
            wqk[k, 32 * r : 32 * r + 32, wx : wx + 32] = wsrc[e]
        else:
            h, sdx = r // 2, r % 2
            wqk[
                k,
                64 * h + 32 * sdx : 64 * h + 32 * sdx + 32,
                wx + 32 * sdx : wx + 32 * sdx + 32,
            ] = wsrc[e]

    bpn = np.zeros((NCORES, 4, O, NQUAD), dtype=mdt)
    bpn[e_core, e_band, :, e_quad] = b[:, 0, :].astype(mdt)
    bpk = bpn.reshape(NCORES, 128, NQUAD)

    plan = (Q, X, TOTW, k32, wqX)
    return plan, order, (k_tok, r_tok, col_tok), (xt8, xt16), wqk, bpk


def _unpack(results, tok_addr, order):
    k_tok, r_tok, col_tok = tok_addr
    ot = np.stack([results[k]["ot"] for k in range(NCORES)])  # [k, 128, TOTW]
    ot4 = ot.reshape(NCORES, 4, O, -1)  # [k, r, o, col]
    out = np.empty((N_TOK, O), dtype=np.float32)
    out[order] = ot4[k_tok, r_tok, :, col_tok]
    return out


def _prepare(x, inds, w, b):
    """Pack inputs and return (nc, in_maps, tok_addr, order)."""
    plan, order, tok_addr, (xt8, xt16), wqk, bpk = _pack(x, inds, w, b)
    Q, X, TOTW, k32, wqX = plan
    key = (
        MM_DT,
        OT_DT,
        N_WARM,
        WARM_N,
        STORE_GPSIMD_N,
        SPLIT_LAST,
        INT8_GROUPS,
        GROUP_ORDER,
        K32_GROUPS,
        Q.tobytes(),
    )
    nc = _programs.get(key)
    if nc is None:
        nc = _build(Q, X, TOTW, k32, wqX)
        _programs[key] = nc
    in_maps = [
        {"xt8": xt8[k], "xt16": xt16[k], "wq": wqk[k], "bp": bpk[k]}
        for k in range(NCORES)
    ]
    return nc, in_maps, tok_addr, order


def kernel(input, inds, w, b):
    x = np.ascontiguousarray(np.asarray(input, dtype=np.float32))
    inds = np.asarray(inds, dtype=np.int32)
    w = np.ascontiguousarray(np.asarray(w, dtype=np.float32))
    b = np.ascontiguousarray(np.asarray(b, dtype=np.float32))
    assert x.shape == (N_TOK, F) and inds.shape == (N_TOK,)
    assert w.shape == (E, F, O) and b.shape == (E, 1, O)

    try:
        nc, in_maps, tok_addr, order = _prepare(x, inds, w, b)
    except _CapacityOverflow:
        return (np.einsum("ni,nio->no", x, w[inds]) + b[inds, 0]).astype(np.float32)

    res = run_bass_kernel_spmd(nc, in_maps, list(range(NCORES)))
    return _unpack(res.results, tok_addr, order)


def last_program():
    """The most recently compiled Bass program (for profiling in test.py)."""
    return next(iter(_programs.values())) if _programs else None

